# revision 8
# baseline (speedup 1.0000x reference)
"""Trainium2 Bass kernel for the ACTP 2-layer-LSTM + MLP rollout model.

Strategy: pure data parallel across 8 NeuronCores (batch 4096 -> 512/core),
weights replicated.  All on-chip tensors are feature-major [feat, batch] so
the time recurrence needs no transposes: matmuls are out[M,N] = W_T[K,M].T @
x[K,N] with the batch as the moving free dim (N=512), gate activations are
batched reads across PSUM banks, and every concat in the model becomes extra
K-chunk matmuls accumulating into the same PSUM bank.  Biases are folded in
as "ones-row" augmented K-chunks.  The tiled(act,state) input of LSTM2
collapses to a K=13 matmul (sum the 4 tiled repeats into the weights).

Only tactiles[0:10] is ever read (the model feeds back its own output after
the context window), so device I/O is tiny.  Host does all transposes.
"""

import os
import sys
import functools

sys.path.insert(0, "/opt/trn_rl_repo")

import numpy as np
import ml_dtypes

import concourse.bass as bass
from concourse import bacc
import concourse.tile as tile
from concourse import mybir
from concourse.bass_utils import run_bass_kernel_spmd

# model dims
T = 120
B = 4096
F = 48   # tactile feature size
A = 6    # action dim
H = 200  # LSTM hidden
CTX = 10
NSTEP = T - 1            # 119 scan steps
NOUT = NSTEP - (CTX - 1)  # 110 outputs
NCORES = 8
BL = B // NCORES         # 512 per-core batch
HC = 100                 # H partition chunk
G4 = 4 * H               # 800 gate rows

COMPUTE_BF16 = True

LAST_RESULT = None  # BassKernelResults of the most recent run (for test.py)

Tanh = mybir.ActivationFunctionType.Tanh
Sigmoid = mybir.ActivationFunctionType.Sigmoid


def _dt():
    return mybir.dt.bfloat16 if COMPUTE_BF16 else mybir.dt.float32


def _npdt():
    return ml_dtypes.bfloat16 if COMPUTE_BF16 else np.float32


def _build_nc():
    nc = bacc.Bacc()
    dt = _dt()
    f32 = mybir.dt.float32

    # ---- DRAM parameters (per-core shards / replicated weights) ----
    tact = nc.declare_dram_parameter("tact", [F, CTX, BL], dt, isOutput=False)
    acts = nc.declare_dram_parameter("acts", [A, NSTEP, BL], dt, isOutput=False)
    # statones: rows 0..5 = state (actions[0] transposed), row 6 = ones
    statones = nc.declare_dram_parameter("statones", [A + 1, BL], dt, isOutput=False)

    w1x = nc.declare_dram_parameter("w1x", [F, G4], dt, isOutput=False)
    w1ha = nc.declare_dram_parameter("w1ha", [HC, G4], dt, isOutput=False)
    w1hb = nc.declare_dram_parameter("w1hb", [HC + 1, G4], dt, isOutput=False)
    w2a6 = nc.declare_dram_parameter("w2a6", [A, G4], dt, isOutput=False)
    w2s7 = nc.declare_dram_parameter("w2s7", [A + 1, G4], dt, isOutput=False)
    w2h1a = nc.declare_dram_parameter("w2h1a", [HC, G4], dt, isOutput=False)
    w2h1b = nc.declare_dram_parameter("w2h1b", [HC, G4], dt, isOutput=False)
    w2h2a = nc.declare_dram_parameter("w2h2a", [HC, G4], dt, isOutput=False)
    w2h2b = nc.declare_dram_parameter("w2h2b", [HC, G4], dt, isOutput=False)
    w3x = nc.declare_dram_parameter("w3x", [F, H], dt, isOutput=False)
    w3ha = nc.declare_dram_parameter("w3ha", [HC, H], dt, isOutput=False)
    w3hb = nc.declare_dram_parameter("w3hb", [HC + 1, H], dt, isOutput=False)
    w4a = nc.declare_dram_parameter("w4a", [HC, F], dt, isOutput=False)
    w4b = nc.declare_dram_parameter("w4b", [HC + 1, F], dt, isOutput=False)

    out = nc.declare_dram_parameter("out", [NOUT, F, BL], f32, isOutput=True)

    from contextlib import ExitStack

    with tile.TileContext(nc) as tc, ExitStack() as ctx:
        # ---- pools ----
        wpool = ctx.enter_context(tc.tile_pool(name="wpool", bufs=1))
        stp = ctx.enter_context(tc.tile_pool(name="stp", bufs=1))
        sp = ctx.enter_context(tc.tile_pool(name="sp", bufs=2))
        op = ctx.enter_context(tc.tile_pool(name="op", bufs=4))
        pp = ctx.enter_context(tc.tile_pool(name="pp", bufs=2, space="PSUM"))

        # ---- weights to SBUF (once) ----
        W1X = wpool.tile([F, G4], dt, name="W1X")
        W1HA = wpool.tile([HC, G4], dt, name="W1HA")
        W1HB = wpool.tile([HC + 1, G4], dt, name="W1HB")
        W2A6 = wpool.tile([A, G4], dt, name="W2A6")
        W2S7 = wpool.tile([A + 1, G4], dt, name="W2S7")
        W2H1A = wpool.tile([HC, G4], dt, name="W2H1A")
        W2H1B = wpool.tile([HC, G4], dt, name="W2H1B")
        W2H2A = wpool.tile([HC, G4], dt, name="W2H2A")
        W2H2B = wpool.tile([HC, G4], dt, name="W2H2B")
        W3X = wpool.tile([F, H], dt, name="W3X")
        W3HA = wpool.tile([HC, H], dt, name="W3HA")
        W3HB = wpool.tile([HC + 1, H], dt, name="W3HB")
        W4A = wpool.tile([HC, F], dt, name="W4A")
        W4B = wpool.tile([HC + 1, F], dt, name="W4B")
        for sb, dr in [
            (W1X, w1x), (W1HA, w1ha), (W1HB, w1hb), (W2A6, w2a6), (W2S7, w2s7),
            (W2H1A, w2h1a), (W2H1B, w2h1b), (W2H2A, w2h2a), (W2H2B, w2h2b),
            (W3X, w3x), (W3HA, w3ha), (W3HB, w3hb), (W4A, w4a), (W4B, w4b),
        ]:
            nc.sync.dma_start(out=sb, in_=dr[:, :])

        # ---- persistent state ----
        # h tiles carry an extra "ones" partition row (101) used as the
        # bias K-chunk by the consumers that want the bias folded in.
        h1 = stp.tile([HC + 1, 2, BL], dt, name="h1")
        h2 = stp.tile([HC + 1, 2, BL], dt, name="h2")
        o3 = stp.tile([HC + 1, 2, BL], dt, name="o3")
        c1 = stp.tile([HC, 2, BL], mybir.dt.float32, name="c1")
        c2 = stp.tile([HC, 2, BL], mybir.dt.float32, name="c2")
        x1 = stp.tile([F, BL], dt, name="x1")
        TACT = stp.tile([F, CTX, BL], dt, name="TACT")
        ACTS = stp.tile([A, NSTEP, BL], dt, name="ACTS")
        ST7 = stp.tile([A + 1, BL], dt, name="ST7")
        nc.sync.dma_start(out=TACT, in_=tact[:, :, :])
        nc.sync.dma_start(out=ACTS, in_=acts[:, :, :])
        nc.sync.dma_start(out=ST7, in_=statones[:, :])

        nc.vector.memset(h1[0:HC, :, :], 0.0)
        nc.vector.memset(h2[0:HC, :, :], 0.0)
        nc.vector.memset(c1, 0.0)
        nc.vector.memset(c2, 0.0)
        # ones rows via DMA from statones row 6 (avoids partition-offset engine ops)
        nc.sync.dma_start(out=h1[HC:HC + 1, 1, :], in_=statones[A:A + 1, :])
        nc.sync.dma_start(out=h2[HC:HC + 1, 1, :], in_=statones[A:A + 1, :])
        nc.sync.dma_start(out=o3[HC:HC + 1, 1, :], in_=statones[A:A + 1, :])

        h1a = h1[0:HC, 0, :]
        h1b = h1[0:HC, 1, :]
        h1b_aug = h1[0:HC + 1, 1, :]
        h2a = h2[0:HC, 0, :]
        h2b = h2[0:HC, 1, :]
        h2b_aug = h2[0:HC + 1, 1, :]

        def lstm_cell(gA, gB, c, h, tag):
            """gates [i0 i1 f0 f1] in gA, [o0 o1 g0 g1] in gB -> update c, h."""
            sg_if = sp.tile([HC, 4, BL], dt, name=f"sgif{tag}", tag="sgif")
            gt = sp.tile([HC, 2, BL], dt, name=f"gt{tag}", tag="gt")
            sg_o = sp.tile([HC, 2, BL], dt, name=f"sgo{tag}", tag="sgo")
            nc.scalar.activation(sg_if, gA[:, 0:4, :], Sigmoid)
            nc.scalar.activation(gt, gB[:, 2:4, :], Tanh)
            nc.scalar.activation(sg_o, gB[:, 0:2, :], Sigmoid)
            ig = sp.tile([HC, 2, BL], dt, name=f"ig{tag}", tag="ig")
            fm = sp.tile([HC, 2, BL], mybir.dt.float32, name=f"fm{tag}", tag="fm")
            nc.vector.tensor_mul(ig, sg_if[:, 0:2, :], gt)
            nc.vector.tensor_mul(fm, sg_if[:, 2:4, :], c)
            nc.vector.tensor_add(c, fm, ig)
            tch = sp.tile([HC, 2, BL], dt, name=f"tch{tag}", tag="tch")
            nc.scalar.activation(tch, c, Tanh)
            nc.vector.tensor_mul(h[0:HC, :, :], sg_o, tch)

        for t in range(NSTEP):
            x1_rhs = TACT[:, t, :] if t < CTX else x1
            act_rhs = ACTS[:, t, :]

            # ---- LSTM1 gates ----
            g1A = pp.tile([HC, 4, BL], mybir.dt.float32, name=f"g1A_{t}", tag="g")
            g1B = pp.tile([HC, 4, BL], mybir.dt.float32, name=f"g1B_{t}", tag="g")
            for m in range(8):
                gp = g1A if m < 4 else g1B
                ps = gp[:, m % 4, :]
                ws = slice(m * HC, (m + 1) * HC)
                nc.tensor.matmul(ps, W1HA[:, ws], h1a, start=True, stop=False)
                nc.tensor.matmul(ps, W1HB[:, ws], h1b_aug, start=False, stop=False)
                nc.tensor.matmul(ps, W1X[:, ws], x1_rhs, start=False, stop=True)
            lstm_cell(g1A, g1B, c1, h1, f"1_{t}")

            # ---- LSTM2 gates ----
            g2A = pp.tile([HC, 4, BL], mybir.dt.float32, name=f"g2A_{t}", tag="g")
            g2B = pp.tile([HC, 4, BL], mybir.dt.float32, name=f"g2B_{t}", tag="g")
            for m in range(8):
                gp = g2A if m < 4 else g2B
                ps = gp[:, m % 4, :]
                ws = slice(m * HC, (m + 1) * HC)
                nc.tensor.matmul(ps, W2H2A[:, ws], h2a, start=True, stop=False)
                nc.tensor.matmul(ps, W2H2B[:, ws], h2b, start=False, stop=False)
                nc.tensor.matmul(ps, W2S7[:, ws], ST7, start=False, stop=False)
                nc.tensor.matmul(ps, W2A6[:, ws], act_rhs, start=False, stop=False)
                nc.tensor.matmul(ps, W2H1A[:, ws], h1a, start=False, stop=False)
                nc.tensor.matmul(ps, W2H1B[:, ws], h1b, start=False, stop=True)
            lstm_cell(g2A, g2B, c2, h2, f"2_{t}")

            # ---- MLP head (only needed from t = CTX-1 on) ----
            if t >= CTX - 1:
                fcp = pp.tile([HC, 3, BL], mybir.dt.float32, name=f"fcp_{t}", tag="g")
                for m in range(2):
                    ps = fcp[:, m, :]
                    ws = slice(m * HC, (m + 1) * HC)
                    nc.tensor.matmul(ps, W3X[:, ws], x1_rhs, start=True, stop=False)
                    nc.tensor.matmul(ps, W3HA[:, ws], h2a, start=False, stop=False)
                    nc.tensor.matmul(ps, W3HB[:, ws], h2b_aug, start=False, stop=True)
                nc.scalar.activation(o3[0:HC, :, :], fcp[:, 0:2, :], Tanh)
                p4 = fcp[0:F, 2, :]
                nc.tensor.matmul(p4, W4A, o3[0:HC, 0, :], start=True, stop=False)
                nc.tensor.matmul(p4, W4B, o3[0:HC + 1, 1, :], start=False, stop=True)
                if t < NSTEP - 1:
                    # feedback: next step's input
                    nc.scalar.activation(x1, p4, Tanh)
                stg = op.tile([F, BL], mybir.dt.float32, name=f"stg_{t}", tag="stg")
                nc.scalar.activation(stg, p4, Tanh)
                nc.gpsimd.dma_start(out=out[t - (CTX - 1)], in_=stg)

    nc.finalize()
    return nc


@functools.lru_cache(maxsize=1)
def _get_nc():
    return _build_nc()


def _prep_weights(W_ih1, W_hh1, b_ih1, b_hh1, W_ih2, W_hh2, b_ih2, b_hh2,
                  fc1_w, fc1_b, fc2_w, fc2_b):
    npdt = _npdt()
    # gate rows reordered [i, f, o, g] so chunk order is [i0 i1 f0 f1 o0 o1 g0 g1]
    perm = np.concatenate([np.arange(0, 200), np.arange(200, 400),
                           np.arange(600, 800), np.arange(400, 600)])
    W1p = np.asarray(W_ih1)[perm]          # [800, 48]
    W1hp = np.asarray(W_hh1)[perm]         # [800, 200]
    b1p = (np.asarray(b_ih1) + np.asarray(b_hh1))[perm]
    W2p = np.asarray(W_ih2)[perm]          # [800, 248]
    W2hp = np.asarray(W_hh2)[perm]         # [800, 200]
    b2p = (np.asarray(b_ih2) + np.asarray(b_hh2))[perm]
    Wt = W2p[:, 200:248]
    W2eff = Wt[:, 0:12] + Wt[:, 12:24] + Wt[:, 24:36] + Wt[:, 36:48]  # [800, 12]
    fc1_w = np.asarray(fc1_w); fc1_b = np.asarray(fc1_b)
    fc2_w = np.asarray(fc2_w); fc2_b = np.asarray(fc2_b)

    def c(x):
        return np.ascontiguousarray(x).astype(npdt)

    return {
        "w1x": c(W1p.T),                                              # [48, 800]
        "w1ha": c(W1hp[:, 0:100].T),                                  # [100, 800]
        "w1hb": c(np.concatenate([W1hp[:, 100:200].T, b1p[None, :]], 0)),
        "w2a6": c(W2eff[:, 0:6].T),                                   # [6, 800]
        "w2s7": c(np.concatenate([W2eff[:, 6:12].T, b2p[None, :]], 0)),  # [7, 800]
        "w2h1a": c(W2p[:, 0:100].T),
        "w2h1b": c(W2p[:, 100:200].T),
        "w2h2a": c(W2hp[:, 0:100].T),
        "w2h2b": c(W2hp[:, 100:200].T),
        "w3x": c(fc1_w[:, 200:248].T),                                # [48, 200]
        "w3ha": c(fc1_w[:, 0:100].T),
        "w3hb": c(np.concatenate([fc1_w[:, 100:200].T, fc1_b[None, :]], 0)),
        "w4a": c(fc2_w[:, 0:100].T),                                  # [100, 48]
        "w4b": c(np.concatenate([fc2_w[:, 100:200].T, fc2_b[None, :]], 0)),
    }


def kernel(tactiles, actions, W_ih1, W_hh1, b_ih1, b_hh1,
           W_ih2, W_hh2, b_ih2, b_hh2, fc1_w, fc1_b, fc2_w, fc2_b):
    global LAST_RESULT
    npdt = _npdt()
    tactiles = np.asarray(tactiles)
    actions = np.asarray(actions)

    wmap = _prep_weights(W_ih1, W_hh1, b_ih1, b_hh1, W_ih2, W_hh2, b_ih2, b_hh2,
                         fc1_w, fc1_b, fc2_w, fc2_b)

    in_maps = []
    for i in range(NCORES):
        s = slice(i * BL, (i + 1) * BL)
        tact_T = np.ascontiguousarray(
            tactiles[0:CTX, s, :].transpose(2, 0, 1)).astype(npdt)   # [48, 10, BL]
        acts_T = np.ascontiguousarray(
            actions[1:T, s, :].transpose(2, 0, 1)).astype(npdt)      # [6, 119, BL]
        statones = np.concatenate(
            [actions[0, s, :].T, np.ones((1, BL), np.float32)], 0).astype(npdt)
        m = {"tact": tact_T, "acts": acts_T, "statones": statones}
        m.update(wmap)
        in_maps.append(m)

    nc = _get_nc()
    res = run_bass_kernel_spmd(nc, in_maps, core_ids=list(range(NCORES)))
    LAST_RESULT = res

    outs = [np.asarray(r["out"], dtype=np.float32) for r in res.results]
    # [NOUT, F, BL] per core -> [NOUT, B, F]
    full = np.concatenate([o.transpose(0, 2, 1) for o in outs], axis=1)
    return np.ascontiguousarray(full)


# revision 9
# speedup vs baseline: 1.6736x; 1.6736x over previous
"""Trainium2 Bass kernel for the ACTP 2-layer-LSTM + MLP rollout model.

Strategy: pure data parallel across 8 NeuronCores (batch 4096 -> 512/core),
weights replicated.  All on-chip tensors are feature-major [feat, batch] so
the time recurrence needs no transposes: matmuls are out[M,N] = W_T[K,M].T @
x[K,N] with the batch as the moving free dim (N=512), gate activations are
batched reads across PSUM banks, and every concat in the model becomes extra
K-chunk matmuls accumulating into the same PSUM bank.  Biases ride in padded
"ones-row" K-chunks (gates, fc1) or the activation bias operand (fc2).  The
tiled(act,state) input of LSTM2 collapses algebraically into a single padded
K=100 chunk.  ALL matmul K-chunks are padded to K=100: matmuls with K <= ~64
in the stream permanently block the PE HAM clock-gate from reaching 2.4 GHz
(measured: mixing K=48 or K=13 pins every matmul at cold ~512ns cadence).

Only tactiles[0:10] is ever read (the model feeds back its own output after
the context window), so device I/O is tiny.  Host does all transposes.
"""

import os
import sys
import functools

sys.path.insert(0, "/opt/trn_rl_repo")

import numpy as np
import ml_dtypes

import concourse.bass as bass
from concourse import bacc
import concourse.tile as tile
from concourse import mybir
from concourse.bass_utils import run_bass_kernel_spmd

# model dims
T = 120
B = 4096
F = 48   # tactile feature size
A = 6    # action dim
H = 200  # LSTM hidden
CTX = 10
NSTEP = T - 1            # 119 scan steps
NOUT = NSTEP - (CTX - 1)  # 110 outputs
NCORES = 8
BL = B // NCORES         # 512 per-core batch
HC = 100                 # H partition chunk (also the universal matmul K)
G4 = 4 * H               # 800 gate rows

COMPUTE_BF16 = True

LAST_RESULT = None  # BassKernelResults of the most recent run (for test.py)

Tanh = mybir.ActivationFunctionType.Tanh
Sigmoid = mybir.ActivationFunctionType.Sigmoid


def _dt():
    return mybir.dt.bfloat16 if COMPUTE_BF16 else mybir.dt.float32


def _npdt():
    return ml_dtypes.bfloat16 if COMPUTE_BF16 else np.float32


def _build_nc():
    nc = bacc.Bacc()
    dt = _dt()
    f32 = mybir.dt.float32

    # ---- DRAM parameters (per-core shards / replicated weights) ----
    # tact: [100, CTX, BL]: rows 0..47 tactile features, row 48 ones, rest 0
    tact = nc.declare_dram_parameter("tact", [HC, CTX, BL], dt, isOutput=False)
    acts = nc.declare_dram_parameter("acts", [A, NSTEP, BL], dt, isOutput=False)
    # statpad: [94, BL]: rows 0..5 state, row 6 ones, rows 7.. zeros
    #   (DMA'd into ast rows 6..99: state at 6..11, ones at 12, zeros 13..99)
    statpad = nc.declare_dram_parameter("statpad", [HC - A, BL], dt, isOutput=False)

    w1x = nc.declare_dram_parameter("w1x", [HC, G4], dt, isOutput=False)
    w1ha = nc.declare_dram_parameter("w1ha", [HC, G4], dt, isOutput=False)
    w1hb = nc.declare_dram_parameter("w1hb", [HC, G4], dt, isOutput=False)
    w2as = nc.declare_dram_parameter("w2as", [HC, G4], dt, isOutput=False)
    w2h1a = nc.declare_dram_parameter("w2h1a", [HC, G4], dt, isOutput=False)
    w2h1b = nc.declare_dram_parameter("w2h1b", [HC, G4], dt, isOutput=False)
    w2h2a = nc.declare_dram_parameter("w2h2a", [HC, G4], dt, isOutput=False)
    w2h2b = nc.declare_dram_parameter("w2h2b", [HC, G4], dt, isOutput=False)
    w3x = nc.declare_dram_parameter("w3x", [HC, H], dt, isOutput=False)
    w3ha = nc.declare_dram_parameter("w3ha", [HC, H], dt, isOutput=False)
    w3hb = nc.declare_dram_parameter("w3hb", [HC, H], dt, isOutput=False)
    w4a = nc.declare_dram_parameter("w4a", [HC, F], dt, isOutput=False)
    w4b = nc.declare_dram_parameter("w4b", [HC, F], dt, isOutput=False)
    b4 = nc.declare_dram_parameter("b4", [F, 1], f32, isOutput=False)

    out = nc.declare_dram_parameter("out", [NOUT, F, BL], f32, isOutput=True)

    from contextlib import ExitStack

    with tile.TileContext(nc) as tc, ExitStack() as ctx:
        # ---- pools ----
        wpool = ctx.enter_context(tc.tile_pool(name="wpool", bufs=1))
        stp = ctx.enter_context(tc.tile_pool(name="stp", bufs=1))
        sp = ctx.enter_context(tc.tile_pool(name="sp", bufs=2))
        op = ctx.enter_context(tc.tile_pool(name="op", bufs=4))
        pp = ctx.enter_context(tc.tile_pool(name="pp", bufs=2, space="PSUM"))

        # ---- weights to SBUF (once) ----
        W1X = wpool.tile([HC, G4], dt, name="W1X")
        W1HA = wpool.tile([HC, G4], dt, name="W1HA")
        W1HB = wpool.tile([HC, G4], dt, name="W1HB")
        W2AS = wpool.tile([HC, G4], dt, name="W2AS")
        W2H1A = wpool.tile([HC, G4], dt, name="W2H1A")
        W2H1B = wpool.tile([HC, G4], dt, name="W2H1B")
        W2H2A = wpool.tile([HC, G4], dt, name="W2H2A")
        W2H2B = wpool.tile([HC, G4], dt, name="W2H2B")
        W3X = wpool.tile([HC, H], dt, name="W3X")
        W3HA = wpool.tile([HC, H], dt, name="W3HA")
        W3HB = wpool.tile([HC, H], dt, name="W3HB")
        W4A = wpool.tile([HC, F], dt, name="W4A")
        W4B = wpool.tile([HC, F], dt, name="W4B")
        B4 = wpool.tile([F, 1], f32, name="B4")
        for sb, dr in [
            (W1X, w1x), (W1HA, w1ha), (W1HB, w1hb), (W2AS, w2as),
            (W2H1A, w2h1a), (W2H1B, w2h1b), (W2H2A, w2h2a), (W2H2B, w2h2b),
            (W3X, w3x), (W3HA, w3ha), (W3HB, w3hb), (W4A, w4a), (W4B, w4b),
            (B4, b4),
        ]:
            nc.sync.dma_start(out=sb, in_=dr[:, :])

        # ---- persistent state ----
        h1 = stp.tile([HC, 2, BL], dt, name="h1")
        h2 = stp.tile([HC, 2, BL], dt, name="h2")
        o3 = stp.tile([HC, 2, BL], dt, name="o3")
        c1 = stp.tile([HC, 2, BL], f32, name="c1")
        c2 = stp.tile([HC, 2, BL], f32, name="c2")
        # x1: rows 0..47 = inp feedback, row 48 = ones (bias ride), 49.. = 0
        x1 = stp.tile([HC, BL], dt, name="x1")
        TACT = stp.tile([HC, CTX, BL], dt, name="TACT")
        ACTS = stp.tile([A, NSTEP, BL], dt, name="ACTS")
        ast0 = stp.tile([HC, BL], dt, name="ast0")
        ast1 = stp.tile([HC, BL], dt, name="ast1")
        nc.sync.dma_start(out=TACT, in_=tact[:, :, :])
        nc.sync.dma_start(out=ACTS, in_=acts[:, :, :])
        # static rows of ast: state(6..11), ones(12), zeros(13..99)
        nc.sync.dma_start(out=ast0[A:HC, :], in_=statpad[:, :])
        nc.sync.dma_start(out=ast1[A:HC, :], in_=statpad[:, :])
        # x1 static rows: ones row at 48 + zero tail (statpad rows 6..57)
        nc.sync.dma_start(out=x1[F:HC, :], in_=statpad[A:A + (HC - F), :])

        nc.vector.memset(h1, 0.0)
        nc.vector.memset(h2, 0.0)
        nc.vector.memset(c1, 0.0)
        nc.vector.memset(c2, 0.0)

        h1a = h1[:, 0, :]
        h1b = h1[:, 1, :]
        h2a = h2[:, 0, :]
        h2b = h2[:, 1, :]

        def lstm_cell(gA, gB, c, h, tag):
            """gates [i0 i1 f0 f1] in gA, [o0 o1 g0 g1] in gB -> update c, h."""
            sg_if = sp.tile([HC, 4, BL], dt, name=f"sgif{tag}", tag="sgif")
            gt = sp.tile([HC, 2, BL], dt, name=f"gt{tag}", tag="gt")
            sg_o = sp.tile([HC, 2, BL], dt, name=f"sgo{tag}", tag="sgo")
            nc.scalar.activation(sg_if, gA[:, 0:4, :], Sigmoid)
            nc.scalar.activation(gt, gB[:, 2:4, :], Tanh)
            nc.scalar.activation(sg_o, gB[:, 0:2, :], Sigmoid)
            ig = sp.tile([HC, 2, BL], dt, name=f"ig{tag}", tag="ig")
            fm = sp.tile([HC, 2, BL], f32, name=f"fm{tag}", tag="fm")
            nc.vector.tensor_mul(ig, sg_if[:, 0:2, :], gt)
            nc.vector.tensor_mul(fm, sg_if[:, 2:4, :], c)
            nc.vector.tensor_add(c, fm, ig)
            tch = sp.tile([HC, 2, BL], dt, name=f"tch{tag}", tag="tch")
            nc.scalar.activation(tch, c, Tanh)
            nc.vector.tensor_mul(h, sg_o, tch)

        for t in range(NSTEP):
            x1_rhs = TACT[:, t, :] if t < CTX else x1
            ast = ast0 if t % 2 == 0 else ast1
            # refresh the act rows (0..5) for this step; same partition base
            nc.vector.tensor_copy(ast[0:A, :], ACTS[:, t, :])

            # ---- LSTM1 gates ----
            g1A = pp.tile([HC, 4, BL], f32, name=f"g1A_{t}", tag="g")
            g1B = pp.tile([HC, 4, BL], f32, name=f"g1B_{t}", tag="g")
            for m in range(8):
                gp = g1A if m < 4 else g1B
                ps = gp[:, m % 4, :]
                ws = slice(m * HC, (m + 1) * HC)
                nc.tensor.matmul(ps, W1HA[:, ws], h1a, start=True, stop=False)
                nc.tensor.matmul(ps, W1HB[:, ws], h1b, start=False, stop=False)
                nc.tensor.matmul(ps, W1X[:, ws], x1_rhs, start=False, stop=True)
            lstm_cell(g1A, g1B, c1, h1, f"1_{t}")

            # ---- LSTM2 gates ----
            g2A = pp.tile([HC, 4, BL], f32, name=f"g2A_{t}", tag="g")
            g2B = pp.tile([HC, 4, BL], f32, name=f"g2B_{t}", tag="g")
            for m in range(8):
                gp = g2A if m < 4 else g2B
                ps = gp[:, m % 4, :]
                ws = slice(m * HC, (m + 1) * HC)
                nc.tensor.matmul(ps, W2H2A[:, ws], h2a, start=True, stop=False)
                nc.tensor.matmul(ps, W2H2B[:, ws], h2b, start=False, stop=False)
                nc.tensor.matmul(ps, W2AS[:, ws], ast, start=False, stop=False)
                nc.tensor.matmul(ps, W2H1A[:, ws], h1a, start=False, stop=False)
                nc.tensor.matmul(ps, W2H1B[:, ws], h1b, start=False, stop=True)
            lstm_cell(g2A, g2B, c2, h2, f"2_{t}")

            # ---- MLP head (only needed from t = CTX-1 on) ----
            if t >= CTX - 1:
                fcp = pp.tile([HC, 3, BL], f32, name=f"fcp_{t}", tag="g")
                for m in range(2):
                    ps = fcp[:, m, :]
                    ws = slice(m * HC, (m + 1) * HC)
                    nc.tensor.matmul(ps, W3X[:, ws], x1_rhs, start=True, stop=False)
                    nc.tensor.matmul(ps, W3HA[:, ws], h2a, start=False, stop=False)
                    nc.tensor.matmul(ps, W3HB[:, ws], h2b, start=False, stop=True)
                nc.scalar.activation(o3, fcp[:, 0:2, :], Tanh)
                p4 = fcp[0:F, 2, :]
                nc.tensor.matmul(p4, W4A, o3[:, 0, :], start=True, stop=False)
                nc.tensor.matmul(p4, W4B, o3[:, 1, :], start=False, stop=True)
                if t < NSTEP - 1:
                    # feedback: next step's input (fc2 bias via ACT bias operand)
                    nc.scalar.activation(x1[0:F, :], p4, Tanh, bias=B4)
                stg = op.tile([F, BL], f32, name=f"stg_{t}", tag="stg")
                nc.scalar.activation(stg, p4, Tanh, bias=B4)
                nc.gpsimd.dma_start(out=out[t - (CTX - 1)], in_=stg)

    nc.finalize()
    return nc


@functools.lru_cache(maxsize=1)
def _get_nc():
    return _build_nc()


def _prep_weights(W_ih1, W_hh1, b_ih1, b_hh1, W_ih2, W_hh2, b_ih2, b_hh2,
                  fc1_w, fc1_b, fc2_w, fc2_b):
    npdt = _npdt()
    # gate rows reordered [i, f, o, g] so chunk order is [i0 i1 f0 f1 o0 o1 g0 g1]
    perm = np.concatenate([np.arange(0, 200), np.arange(200, 400),
                           np.arange(600, 800), np.arange(400, 600)])
    W1p = np.asarray(W_ih1)[perm]          # [800, 48]
    W1hp = np.asarray(W_hh1)[perm]         # [800, 200]
    b1p = (np.asarray(b_ih1) + np.asarray(b_hh1))[perm]
    W2p = np.asarray(W_ih2)[perm]          # [800, 248]
    W2hp = np.asarray(W_hh2)[perm]         # [800, 200]
    b2p = (np.asarray(b_ih2) + np.asarray(b_hh2))[perm]
    Wt = W2p[:, 200:248]
    W2eff = Wt[:, 0:12] + Wt[:, 12:24] + Wt[:, 24:36] + Wt[:, 36:48]  # [800, 12]
    fc1_w = np.asarray(fc1_w); fc1_b = np.asarray(fc1_b)
    fc2_w = np.asarray(fc2_w); fc2_b = np.asarray(fc2_b)

    def c(x):
        return np.ascontiguousarray(x).astype(npdt)

    def padK(x):
        k, m = x.shape
        z = np.zeros((HC, m), x.dtype)
        z[:k] = x
        return z

    # x-chunk weights: rows 0..47 = input features, row 48 = bias, rest 0
    w1x = padK(np.concatenate([W1p.T, b1p[None, :]], 0))          # [100, 800]
    w3x = padK(np.concatenate([fc1_w[:, 200:248].T, fc1_b[None, :]], 0))
    # act/state chunk: rows 0..5 act, 6..11 state, row 12 bias, rest 0
    w2as = padK(np.concatenate([W2eff.T, b2p[None, :]], 0))       # [100, 800]

    return {
        "w1x": c(w1x),
        "w1ha": c(W1hp[:, 0:100].T),
        "w1hb": c(W1hp[:, 100:200].T),
        "w2as": c(w2as),
        "w2h1a": c(W2p[:, 0:100].T),
        "w2h1b": c(W2p[:, 100:200].T),
        "w2h2a": c(W2hp[:, 0:100].T),
        "w2h2b": c(W2hp[:, 100:200].T),
        "w3x": c(w3x),
        "w3ha": c(fc1_w[:, 0:100].T),
        "w3hb": c(fc1_w[:, 100:200].T),
        "w4a": c(fc2_w[:, 0:100].T),
        "w4b": c(fc2_w[:, 100:200].T),
        "b4": np.ascontiguousarray(fc2_b[:, None]).astype(np.float32),
    }


def kernel(tactiles, actions, W_ih1, W_hh1, b_ih1, b_hh1,
           W_ih2, W_hh2, b_ih2, b_hh2, fc1_w, fc1_b, fc2_w, fc2_b):
    global LAST_RESULT
    npdt = _npdt()
    tactiles = np.asarray(tactiles)
    actions = np.asarray(actions)

    wmap = _prep_weights(W_ih1, W_hh1, b_ih1, b_hh1, W_ih2, W_hh2, b_ih2, b_hh2,
                         fc1_w, fc1_b, fc2_w, fc2_b)

    in_maps = []
    for i in range(NCORES):
        s = slice(i * BL, (i + 1) * BL)
        # tact: [100, CTX, BL] with row 48 = ones (bias ride), rest 0
        tt = np.zeros((HC, CTX, BL), np.float32)
        tt[0:F] = tactiles[0:CTX, s, :].transpose(2, 0, 1)
        tt[F] = 1.0
        acts_T = np.ascontiguousarray(
            actions[1:T, s, :].transpose(2, 0, 1)).astype(npdt)      # [6, 119, BL]
        # statpad rows (land at ast rows 6..99): state(6), ones(1), zeros
        sp_ = np.zeros((HC - A, BL), np.float32)
        sp_[0:A] = actions[0, s, :].T
        sp_[A] = 1.0
        m = {"tact": tt.astype(npdt), "acts": acts_T,
             "statpad": sp_.astype(npdt)}
        m.update(wmap)
        in_maps.append(m)

    nc = _get_nc()
    res = run_bass_kernel_spmd(nc, in_maps, core_ids=list(range(NCORES)))
    LAST_RESULT = res

    outs = [np.asarray(r["out"], dtype=np.float32) for r in res.results]
    # [NOUT, F, BL] per core -> [NOUT, B, F]
    full = np.concatenate([o.transpose(0, 2, 1) for o in outs], axis=1)
    return np.ascontiguousarray(full)


# revision 10
# speedup vs baseline: 2.0403x; 1.2191x over previous
"""Trainium2 Bass kernel for the ACTP 2-layer-LSTM + MLP rollout model.

Strategy: pure data parallel across 8 NeuronCores (batch 4096 -> 512/core),
weights replicated.  All on-chip tensors are feature-major [feat, batch] so
the time recurrence needs no transposes: matmuls are out[M,N] = W_T[K,M].T @
x[K,N] with the batch as the moving free dim (N=512), gate activations are
batched reads across PSUM banks, and every concat in the model becomes extra
K-chunk matmuls accumulating into the same PSUM bank.  Biases ride in padded
"ones-row" K-chunks (gates, fc1) or the activation bias operand (fc2).  The
tiled(act,state) input of LSTM2 collapses algebraically into a single padded
K=100 chunk.  ALL matmul K-chunks are padded to K=100: matmuls with K <= ~64
in the stream permanently block the PE HAM clock-gate from reaching 2.4 GHz
(measured: mixing K=48 or K=13 pins every matmul at cold ~512ns cadence).

Only tactiles[0:10] is ever read (the model feeds back its own output after
the context window), so device I/O is tiny.  Host does all transposes.
"""

import os
import sys
import functools

sys.path.insert(0, "/opt/trn_rl_repo")

import numpy as np
import ml_dtypes

import concourse.bass as bass
from concourse import bacc
import concourse.tile as tile
from concourse import mybir
from concourse.bass_utils import run_bass_kernel_spmd

# model dims
T = 120
B = 4096
F = 48   # tactile feature size
A = 6    # action dim
H = 200  # LSTM hidden
CTX = 10
NSTEP = T - 1            # 119 scan steps
NOUT = NSTEP - (CTX - 1)  # 110 outputs
NCORES = 8
BL = B // NCORES         # 512 per-core batch
HC = 100                 # H partition chunk (also the universal matmul K)
G4 = 4 * H               # 800 gate rows

COMPUTE_BF16 = True

LAST_RESULT = None  # BassKernelResults of the most recent run (for test.py)

Tanh = mybir.ActivationFunctionType.Tanh
Sigmoid = mybir.ActivationFunctionType.Sigmoid


def _dt():
    return mybir.dt.bfloat16 if COMPUTE_BF16 else mybir.dt.float32


def _npdt():
    return ml_dtypes.bfloat16 if COMPUTE_BF16 else np.float32


def _build_nc():
    nc = bacc.Bacc()
    dt = _dt()
    f32 = mybir.dt.float32

    # ---- DRAM parameters (per-core shards / replicated weights) ----
    # tact: [100, CTX, BL]: rows 0..47 tactile features, row 48 ones, rest 0
    tact = nc.declare_dram_parameter("tact", [HC, CTX, BL], dt, isOutput=False)
    acts = nc.declare_dram_parameter("acts", [A, NSTEP, BL], dt, isOutput=False)
    # statpad: [94, BL]: rows 0..5 state, row 6 ones, rows 7.. zeros
    #   (DMA'd into ast rows 6..99: state at 6..11, ones at 12, zeros 13..99)
    statpad = nc.declare_dram_parameter("statpad", [HC - A, BL], dt, isOutput=False)

    w1x = nc.declare_dram_parameter("w1x", [HC, G4], dt, isOutput=False)
    w1ha = nc.declare_dram_parameter("w1ha", [HC, G4], dt, isOutput=False)
    w1hb = nc.declare_dram_parameter("w1hb", [HC, G4], dt, isOutput=False)
    w2as = nc.declare_dram_parameter("w2as", [HC, G4], dt, isOutput=False)
    w2h1a = nc.declare_dram_parameter("w2h1a", [HC, G4], dt, isOutput=False)
    w2h1b = nc.declare_dram_parameter("w2h1b", [HC, G4], dt, isOutput=False)
    w2h2a = nc.declare_dram_parameter("w2h2a", [HC, G4], dt, isOutput=False)
    w2h2b = nc.declare_dram_parameter("w2h2b", [HC, G4], dt, isOutput=False)
    w3x = nc.declare_dram_parameter("w3x", [HC, H], dt, isOutput=False)
    w3ha = nc.declare_dram_parameter("w3ha", [HC, H], dt, isOutput=False)
    w3hb = nc.declare_dram_parameter("w3hb", [HC, H], dt, isOutput=False)
    w4a = nc.declare_dram_parameter("w4a", [HC, F], dt, isOutput=False)
    w4b = nc.declare_dram_parameter("w4b", [HC, F], dt, isOutput=False)
    b4 = nc.declare_dram_parameter("b4", [F, 1], f32, isOutput=False)

    out = nc.declare_dram_parameter("out", [NOUT, F, BL], f32, isOutput=True)

    from contextlib import ExitStack

    with tile.TileContext(nc) as tc, ExitStack() as ctx:
        # ---- pools ----
        wpool = ctx.enter_context(tc.tile_pool(name="wpool", bufs=1))
        stp = ctx.enter_context(tc.tile_pool(name="stp", bufs=1))
        sp = ctx.enter_context(tc.tile_pool(name="sp", bufs=2))
        op = ctx.enter_context(tc.tile_pool(name="op", bufs=4))
        pp = ctx.enter_context(tc.tile_pool(name="pp", bufs=4, space="PSUM"))

        # ---- weights to SBUF (once) ----
        W1X = wpool.tile([HC, G4], dt, name="W1X")
        W1HA = wpool.tile([HC, G4], dt, name="W1HA")
        W1HB = wpool.tile([HC, G4], dt, name="W1HB")
        W2AS = wpool.tile([HC, G4], dt, name="W2AS")
        W2H1A = wpool.tile([HC, G4], dt, name="W2H1A")
        W2H1B = wpool.tile([HC, G4], dt, name="W2H1B")
        W2H2A = wpool.tile([HC, G4], dt, name="W2H2A")
        W2H2B = wpool.tile([HC, G4], dt, name="W2H2B")
        W3X = wpool.tile([HC, H], dt, name="W3X")
        W3HA = wpool.tile([HC, H], dt, name="W3HA")
        W3HB = wpool.tile([HC, H], dt, name="W3HB")
        W4A = wpool.tile([HC, F], dt, name="W4A")
        W4B = wpool.tile([HC, F], dt, name="W4B")
        B4 = wpool.tile([F, 1], f32, name="B4")
        for sb, dr in [
            (W1X, w1x), (W1HA, w1ha), (W1HB, w1hb), (W2AS, w2as),
            (W2H1A, w2h1a), (W2H1B, w2h1b), (W2H2A, w2h2a), (W2H2B, w2h2b),
            (W3X, w3x), (W3HA, w3ha), (W3HB, w3hb), (W4A, w4a), (W4B, w4b),
            (B4, b4),
        ]:
            nc.sync.dma_start(out=sb, in_=dr[:, :])

        # ---- persistent state ----
        h1 = stp.tile([HC, 2, BL], dt, name="h1")
        h2 = stp.tile([HC, 2, BL], dt, name="h2")
        o3 = stp.tile([HC, 2, BL], dt, name="o3")
        c1 = stp.tile([HC, 2, BL], f32, name="c1")
        c2 = stp.tile([HC, 2, BL], f32, name="c2")
        # x1: rows 0..47 = inp feedback, row 48 = ones (bias ride), 49.. = 0
        x1 = stp.tile([HC, BL], dt, name="x1")
        TACT = stp.tile([HC, CTX, BL], dt, name="TACT")
        ACTS = stp.tile([A, NSTEP, BL], dt, name="ACTS")
        ast0 = stp.tile([HC, BL], dt, name="ast0")
        ast1 = stp.tile([HC, BL], dt, name="ast1")
        nc.sync.dma_start(out=TACT, in_=tact[:, :, :])
        nc.sync.dma_start(out=ACTS, in_=acts[:, :, :])
        # static rows of ast: state(6..11), ones(12), zeros(13..99)
        nc.sync.dma_start(out=ast0[A:HC, :], in_=statpad[:, :])
        nc.sync.dma_start(out=ast1[A:HC, :], in_=statpad[:, :])
        # x1 static rows: ones row at 48 + zero tail (statpad rows 6..57)
        nc.sync.dma_start(out=x1[F:HC, :], in_=statpad[A:A + (HC - F), :])

        nc.vector.memset(h1, 0.0)
        nc.vector.memset(h2, 0.0)
        nc.vector.memset(c1, 0.0)
        nc.vector.memset(c2, 0.0)

        h1a = h1[:, 0, :]
        h1b = h1[:, 1, :]
        h2a = h2[:, 0, :]
        h2b = h2[:, 1, :]

        # permuted gate row layout: [i(0:200) f(200:400) o(400:600) g(600:800)]
        COLBASE = {"i": 0, "f": 200, "o": 400, "g": 600}

        def lstm_gates(kchunks, tag):
            """Emit per-gate 2-bank PSUM tiles in order [g, i, f, o] (g first:
            tanh(g) heads the elementwise chain).  kchunks: list of
            (weight_tile, rhs_ap); accumulation runs in list order."""
            P = {}
            for gate in ("g", "i", "f", "o"):
                gp = pp.tile([HC, 2, BL], f32, name=f"P{gate}_{tag}", tag="g")
                for m in range(2):
                    col = COLBASE[gate] + m * HC
                    last = len(kchunks) - 1
                    for j, (W, rhs) in enumerate(kchunks):
                        nc.tensor.matmul(gp[:, m, :], W[:, col:col + HC], rhs,
                                         start=(j == 0), stop=(j == last))
                P[gate] = gp
            return P

        def lstm_cell(P, c, h, tag):
            gt = sp.tile([HC, 2, BL], dt, name=f"gt{tag}", tag="gt")
            sgi = sp.tile([HC, 2, BL], dt, name=f"sgi{tag}", tag="sgi")
            sgf = sp.tile([HC, 2, BL], dt, name=f"sgf{tag}", tag="sgf")
            sgo = sp.tile([HC, 2, BL], dt, name=f"sgo{tag}", tag="sgo")
            nc.scalar.activation(gt, P["g"], Tanh)
            nc.scalar.activation(sgi, P["i"], Sigmoid)
            nc.scalar.activation(sgf, P["f"], Sigmoid)
            nc.scalar.activation(sgo, P["o"], Sigmoid)
            ig = sp.tile([HC, 2, BL], dt, name=f"ig{tag}", tag="ig")
            fm = sp.tile([HC, 2, BL], f32, name=f"fm{tag}", tag="fm")
            nc.vector.tensor_mul(ig, sgi, gt)
            nc.vector.tensor_mul(fm, sgf, c)
            nc.vector.tensor_add(c, fm, ig)
            tch = sp.tile([HC, 2, BL], dt, name=f"tch{tag}", tag="tch")
            nc.scalar.activation(tch, c, Tanh)
            nc.vector.tensor_mul(h, sgo, tch)

        for t in range(NSTEP):
            x1_rhs = TACT[:, t, :] if t < CTX else x1
            ast = ast0 if t % 2 == 0 else ast1
            # refresh the act rows (0..5) for this step; same partition base
            nc.vector.tensor_copy(ast[0:A, :], ACTS[:, t, :])

            # ---- LSTM1 gates ----
            P1 = lstm_gates([(W1HA, h1a), (W1HB, h1b), (W1X, x1_rhs)], f"1_{t}")
            lstm_cell(P1, c1, h1, f"1_{t}")

            # ---- LSTM2 gates ----
            P2 = lstm_gates([(W2H2A, h2a), (W2H2B, h2b), (W2AS, ast),
                             (W2H1A, h1a), (W2H1B, h1b)], f"2_{t}")
            lstm_cell(P2, c2, h2, f"2_{t}")

            # ---- MLP head (only needed from t = CTX-1 on) ----
            if t >= CTX - 1:
                fcp = pp.tile([HC, 2, BL], f32, name=f"fcp_{t}", tag="g")
                for m in range(2):
                    ps = fcp[:, m, :]
                    ws = slice(m * HC, (m + 1) * HC)
                    nc.tensor.matmul(ps, W3X[:, ws], x1_rhs, start=True, stop=False)
                    nc.tensor.matmul(ps, W3HA[:, ws], h2a, start=False, stop=False)
                    nc.tensor.matmul(ps, W3HB[:, ws], h2b, start=False, stop=True)
                nc.scalar.activation(o3, fcp, Tanh)
                f2p = pp.tile([F, BL], f32, name=f"f2p_{t}", tag="g")
                p4 = f2p[:, :]
                nc.tensor.matmul(p4, W4A, o3[:, 0, :], start=True, stop=False)
                nc.tensor.matmul(p4, W4B, o3[:, 1, :], start=False, stop=True)
                if t < NSTEP - 1:
                    # feedback: next step's input (fc2 bias via ACT bias operand)
                    nc.scalar.activation(x1[0:F, :], p4, Tanh, bias=B4)
                stg = op.tile([F, BL], f32, name=f"stg_{t}", tag="stg")
                nc.scalar.activation(stg, p4, Tanh, bias=B4)
                nc.gpsimd.dma_start(out=out[t - (CTX - 1)], in_=stg)

    nc.finalize()
    return nc


@functools.lru_cache(maxsize=1)
def _get_nc():
    return _build_nc()


def _prep_weights(W_ih1, W_hh1, b_ih1, b_hh1, W_ih2, W_hh2, b_ih2, b_hh2,
                  fc1_w, fc1_b, fc2_w, fc2_b):
    npdt = _npdt()
    # gate rows reordered [i, f, o, g] so chunk order is [i0 i1 f0 f1 o0 o1 g0 g1]
    perm = np.concatenate([np.arange(0, 200), np.arange(200, 400),
                           np.arange(600, 800), np.arange(400, 600)])
    W1p = np.asarray(W_ih1)[perm]          # [800, 48]
    W1hp = np.asarray(W_hh1)[perm]         # [800, 200]
    b1p = (np.asarray(b_ih1) + np.asarray(b_hh1))[perm]
    W2p = np.asarray(W_ih2)[perm]          # [800, 248]
    W2hp = np.asarray(W_hh2)[perm]         # [800, 200]
    b2p = (np.asarray(b_ih2) + np.asarray(b_hh2))[perm]
    Wt = W2p[:, 200:248]
    W2eff = Wt[:, 0:12] + Wt[:, 12:24] + Wt[:, 24:36] + Wt[:, 36:48]  # [800, 12]
    fc1_w = np.asarray(fc1_w); fc1_b = np.asarray(fc1_b)
    fc2_w = np.asarray(fc2_w); fc2_b = np.asarray(fc2_b)

    def c(x):
        return np.ascontiguousarray(x).astype(npdt)

    def padK(x):
        k, m = x.shape
        z = np.zeros((HC, m), x.dtype)
        z[:k] = x
        return z

    # x-chunk weights: rows 0..47 = input features, row 48 = bias, rest 0
    w1x = padK(np.concatenate([W1p.T, b1p[None, :]], 0))          # [100, 800]
    w3x = padK(np.concatenate([fc1_w[:, 200:248].T, fc1_b[None, :]], 0))
    # act/state chunk: rows 0..5 act, 6..11 state, row 12 bias, rest 0
    w2as = padK(np.concatenate([W2eff.T, b2p[None, :]], 0))       # [100, 800]

    return {
        "w1x": c(w1x),
        "w1ha": c(W1hp[:, 0:100].T),
        "w1hb": c(W1hp[:, 100:200].T),
        "w2as": c(w2as),
        "w2h1a": c(W2p[:, 0:100].T),
        "w2h1b": c(W2p[:, 100:200].T),
        "w2h2a": c(W2hp[:, 0:100].T),
        "w2h2b": c(W2hp[:, 100:200].T),
        "w3x": c(w3x),
        "w3ha": c(fc1_w[:, 0:100].T),
        "w3hb": c(fc1_w[:, 100:200].T),
        "w4a": c(fc2_w[:, 0:100].T),
        "w4b": c(fc2_w[:, 100:200].T),
        "b4": np.ascontiguousarray(fc2_b[:, None]).astype(np.float32),
    }


def kernel(tactiles, actions, W_ih1, W_hh1, b_ih1, b_hh1,
           W_ih2, W_hh2, b_ih2, b_hh2, fc1_w, fc1_b, fc2_w, fc2_b):
    global LAST_RESULT
    npdt = _npdt()
    tactiles = np.asarray(tactiles)
    actions = np.asarray(actions)

    wmap = _prep_weights(W_ih1, W_hh1, b_ih1, b_hh1, W_ih2, W_hh2, b_ih2, b_hh2,
                         fc1_w, fc1_b, fc2_w, fc2_b)

    in_maps = []
    for i in range(NCORES):
        s = slice(i * BL, (i + 1) * BL)
        # tact: [100, CTX, BL] with row 48 = ones (bias ride), rest 0
        tt = np.zeros((HC, CTX, BL), np.float32)
        tt[0:F] = tactiles[0:CTX, s, :].transpose(2, 0, 1)
        tt[F] = 1.0
        acts_T = np.ascontiguousarray(
            actions[1:T, s, :].transpose(2, 0, 1)).astype(npdt)      # [6, 119, BL]
        # statpad rows (land at ast rows 6..99): state(6), ones(1), zeros
        sp_ = np.zeros((HC - A, BL), np.float32)
        sp_[0:A] = actions[0, s, :].T
        sp_[A] = 1.0
        m = {"tact": tt.astype(npdt), "acts": acts_T,
             "statpad": sp_.astype(npdt)}
        m.update(wmap)
        in_maps.append(m)

    nc = _get_nc()
    res = run_bass_kernel_spmd(nc, in_maps, core_ids=list(range(NCORES)))
    LAST_RESULT = res

    outs = [np.asarray(r["out"], dtype=np.float32) for r in res.results]
    # [NOUT, F, BL] per core -> [NOUT, B, F]
    full = np.concatenate([o.transpose(0, 2, 1) for o in outs], axis=1)
    return np.ascontiguousarray(full)
